# revision 29
# baseline (speedup 1.0000x reference)
"""Newton-Schulz iterative matrix inverse on Trainium2 (Bass/Tile), 8-core SPMD.

Math (per 128x128 matrix W):
    s  = norm1(W) * norminf(W)
    X0 = W^T / s;  X_{k+1} = X_k (2I - W X_k)   [num_iters times]

Transpose-free reformulation: X = W^T q(H) / s with H = W W^T / s and the
symmetric polynomial recurrence Q' = Q (2I - H Q), Q_0 = I, evaluated with
the Horner head p_2 = 4 - 6H + 4H^2 - H^3 (2 matmuls) then NS doubling.

All data is fp16 on SBUF (W is cast during the SWDGE input DMA); PSUM stays
fp32.  The per-matrix factor fs = 2^a/s is folded into scaled copies of the
matmul OPERANDS (vt16 = fs*W^T for H, ws16 = fs*W for the final product) on
the gpsimd engine, so the H and X PSUM results evacuate as plain copies.

W^T (and |W|^T for the norms) are produced by ONE batched XBAR DMA
transpose each per group: with the 3D output AP [c, m, r] the full-2D
transpose equals per-matrix transposes.

Norms: |W| via gpsimd, column sums of |W| and |W^T| via per-matrix PE
matmuls against a ones vector accumulated into a resident PSUM bank, and a
batched finalize (PE transpose + DVE max/mul/reciprocal) every 4 groups
producing fs broadcast columns.

Groups of G=8 matrices share one [128, 8*128] PSUM tile (2 banks), so each
PSUM->SBUF evacuation is a single [128,1024] op.  Evacuations are split
across ACT (plain copies + the first 2I-T via PSUM preload) and DVE
(scalar_tensor_tensor forms); gpsimd does the SBUF-side scaled copies.
Stage emission is skew-interleaved across groups to keep engines busy.
"""

import numpy as np

import concourse.bass as bass
import concourse.mybir as mybir
import concourse.tile as tile
from concourse import bacc, bass_utils

F32 = mybir.dt.float32
F16 = mybir.dt.float16
U16 = mybir.dt.uint16
AF = mybir.ActivationFunctionType
ALU = mybir.AluOpType
AX = mybir.AxisListType

N_CORES = 8
M_PER_CORE = 128          # 64*16 / 8 matrices per core
N = 128                   # matrix dim
A_EXP = 3                 # power-of-2 scale
G = 8                     # matrices per group (two PSUM banks)
N_GROUPS = M_PER_CORE // G
FB = 4                    # groups per norm-finalize batch (32 matrices)
SKEW = 4                  # stage offset between consecutive groups
M1_ACT = 4                # matrices of the first 2I-T evac done on ACT
ABS_ACT = 4               # matrices of |W| done on ACT (rest DVE bitwise)

_nc_cache: dict = {}


def _build(num_iters: int):
    nc = bacc.Bacc("TRN2", target_bir_lowering=False, debug=False,
                   num_devices=N_CORES)

    W_d = nc.dram_tensor("W", [M_PER_CORE, N * N], F32, kind="ExternalInput").ap()
    EYE32_d = nc.dram_tensor("EYE32", [N, N], F32, kind="ExternalInput").ap()
    TWOI_d = nc.dram_tensor("TWOI", [N, G * N], F32, kind="ExternalInput").ap()
    C6A2N_d = nc.dram_tensor("C6A2N", [N, G * N], F32, kind="ExternalInput").ap()
    C4NI_d = nc.dram_tensor("C4NI", [N, G * N], F32, kind="ExternalInput").ap()
    C4AN16_d = nc.dram_tensor("C4AN16", [N, N], F16, kind="ExternalInput").ap()
    C32I16_d = nc.dram_tensor("C32I16", [N, N], F16, kind="ExternalInput").ap()
    CN2I16_d = nc.dram_tensor("CN2I16", [N, G * N], F16, kind="ExternalInput").ap()
    C21A_d = nc.dram_tensor("C21A", [N, G * N], F32, kind="ExternalInput").ap()
    EYENA_d = nc.dram_tensor("EYENA", [N, G * N], F32, kind="ExternalInput").ap()
    X_d = nc.dram_tensor("X", [M_PER_CORE, N * N], F32, kind="ExternalOutput").ap()

    W3 = W_d.rearrange("m (r c) -> m r c", c=N)
    X3 = X_d.rearrange("m (r c) -> m r c", c=N)

    with tile.TileContext(nc) as tc:
        with (
            tc.tile_pool(name="const", bufs=1) as cp,
            tc.tile_pool(name="main", bufs=3) as mp,
            tc.tile_pool(name="psum", bufs=8, space="PSUM") as pp,
        ):
            # ---- constants ----
            hp = tc.high_priority()
            hp.__enter__()
            eye32 = cp.tile([N, N], F32, name="eye32")
            nc.scalar.dma_start(eye32, EYE32_d)
            if num_iters >= 1:
                twoi = None
                if num_iters >= 3:
                    twoi = cp.tile([N, G * N], F32, name="twoi")
                    nc.scalar.dma_start(twoi, TWOI_d)
                if num_iters >= 2:
                    c6a2n = cp.tile([N, G * N], F32, name="c6a2n")
                    c4ni = cp.tile([N, G * N], F32, name="c4ni")
                    c4an16 = cp.tile([N, N], F16, name="c4an16")
                    nc.scalar.dma_start(c6a2n, C6A2N_d)
                    nc.scalar.dma_start(c4ni, C4NI_d)
                    nc.scalar.dma_start(c4an16, C4AN16_d)
                if num_iters > 3:
                    c32i16 = cp.tile([N, N], F16, name="c32i16")
                    cn2i16 = cp.tile([N, G * N], F16, name="cn2i16")
                    nc.scalar.dma_start(c32i16, C32I16_d)
                    nc.scalar.dma_start(cn2i16, CN2I16_d)
                if num_iters == 1:
                    c21a = cp.tile([N, G * N], F32, name="c21a")
                    nc.scalar.dma_start(c21a, C21A_d)
            else:
                eyena = cp.tile([N, G * N], F32, name="eyena")
                nc.scalar.dma_start(eyena, EYENA_d)

            hp.__exit__(None, None, None)
            ones16 = cp.tile([N, 1], F16, name="ones16")
            nc.vector.memset(ones16, 1.0)
            onesa = cp.tile([1, N], F32, name="onesa")
            nc.vector.memset(onesa, float(2.0 ** A_EXP))
            fs_bc = cp.tile([N, M_PER_CORE], F32, name="fs_bc")
            fs_bc16 = cp.tile([N, M_PER_CORE], F16, name="fs_bc16")

            # resident norm PSUM bank:
            #   CS [:,0:128] colsum cols, RS [:,128:256] rowsum cols,
            #   TPc [0:32,256:384], TPr [0:32,384:512] per-batch transposes
            nrm = pp.tile([N, 4 * N], F32, tag="nrm", bufs=1, name="nrm")
            CS = nrm[:, 0:N]
            RS = nrm[:, N:2 * N]
            # second resident bank for the tiny fs-broadcast matmul outputs
            fin = pp.tile([N, 4 * N], F32, tag="fin", bufs=1, name="fin")

            sl = [slice(i * N, (i + 1) * N) for i in range(G)]

            # ---- per-group staged pipeline ----
            def group_stages(g):
                st = {}
                m0 = g * G
                gsl = slice(m0, m0 + G)

                def s_in():
                    # All-resident input tiles + high priority: the SWDGE
                    # desc-gens run on the Pool engine and must not queue
                    # behind the vt/ws tensor ops there.
                    st["w16"] = mp.tile([N, G * N], F16, tag="w16",
                                        bufs=N_GROUPS, name="w16")
                    with tc.high_priority():
                        nc.gpsimd.dma_start(
                            st["w16"].rearrange("p (m c) -> p m c", c=N),
                            W3[m0:m0 + G].rearrange("m r c -> r m c"))

                def s_trn():
                    st["wt16"] = mp.tile([N, G * N], F16, tag="wt16", bufs=8,
                                         name="wt16")
                    nc.sync.dma_start_transpose(
                        st["wt16"].rearrange("p (m r) -> p m r", m=G),
                        st["w16"])

                def s_aw():
                    # |w|: clear the fp16 sign bit; split ACT/DVE for balance
                    st["aw"] = mp.tile([N, G * N], F16, tag="aw", bufs=3,
                                       name="aw")
                    ka = ABS_ACT * N
                    if ABS_ACT > 0:
                        nc.scalar.activation(st["aw"][:, 0:ka],
                                             st["w16"][:, 0:ka], AF.Abs)
                    if ABS_ACT < G:
                        nc.vector.tensor_scalar(
                            st["aw"][:, ka:].bitcast(U16),
                            st["w16"][:, ka:].bitcast(U16),
                            0x7FFF, None, op0=ALU.bitwise_and)

                def s_awt():
                    st["awt"] = mp.tile([N, G * N], F16, tag="awt", bufs=3,
                                        name="awt")
                    nc.sync.dma_start_transpose(
                        st["awt"].rearrange("p (m r) -> p m r", m=G),
                        st["aw"])

                def s_nsum():
                    for i in range(G):
                        nc.tensor.matmul(CS[:, m0 + i:m0 + i + 1],
                                         st["aw"][:, sl[i]], ones16,
                                         start=True, stop=True)
                        nc.tensor.matmul(RS[:, m0 + i:m0 + i + 1],
                                         st["awt"][:, sl[i]], ones16,
                                         start=True, stop=True)

                stages = [s_in, s_trn, s_aw, s_awt, s_nsum]

                # batched norm finalize -> fs_bc columns for FB groups
                if g % FB == FB - 1:
                    b = g // FB
                    bs = slice(b * FB * G, (b + 1) * FB * G)   # 32 matrices
                    nb = FB * G

                    def s_fin1():
                        st["nsb"] = mp.tile([N, 2 * nb], F32, tag="nsb",
                                            bufs=2, name="nsb")
                        nc.scalar.activation(st["nsb"][:, 0:nb], CS[:, bs],
                                             AF.Copy)
                        nc.scalar.activation(st["nsb"][:, nb:2 * nb],
                                             RS[:, bs], AF.Copy)

                    def s_fin2():
                        nc.tensor.transpose(nrm[0:nb, 2 * N:3 * N],
                                            st["nsb"][:, 0:nb], eye32)
                        nc.tensor.transpose(nrm[0:nb, 3 * N:4 * N],
                                            st["nsb"][:, nb:2 * nb], eye32)

                    def s_fin3():
                        st["mxc"] = mp.tile([nb, 1], F32, tag="mxc", bufs=2,
                                            name="mxc")
                        st["mxr"] = mp.tile([nb, 1], F32, tag="mxr", bufs=2,
                                            name="mxr")
                        nc.vector.tensor_reduce(st["mxc"], nrm[0:nb, 2 * N:3 * N],
                                                axis=AX.X, op=ALU.max)
                        nc.vector.tensor_reduce(st["mxr"], nrm[0:nb, 3 * N:4 * N],
                                                axis=AX.X, op=ALU.max)

                    def s_fin4():
                        st["s32"] = mp.tile([nb, 1], F32, tag="s32", bufs=2,
                                            name="s32")
                        nc.vector.tensor_tensor(st["s32"], st["mxc"], st["mxr"],
                                                op=ALU.mult)
                        st["rcp"] = mp.tile([nb, 1], F32, tag="rcp", bufs=2,
                                            name="rcp")
                        nc.vector.reciprocal(st["rcp"], st["s32"])

                    def s_fin5():
                        st["fsT_ps"] = fin[0:1, 0:nb]
                        nc.tensor.matmul(st["fsT_ps"], st["rcp"],
                                         eye32[0:nb, 0:nb],
                                         start=True, stop=True)

                    def s_fin6():
                        st["fsT"] = mp.tile([1, nb], F32, tag="fsTs", bufs=2,
                                            name="fsT")
                        nc.scalar.activation(st["fsT"], st["fsT_ps"], AF.Copy)

                    def s_fin7():
                        st["bc_ps"] = fin[:, N:N + nb]
                        nc.tensor.matmul(st["bc_ps"], onesa, st["fsT"],
                                         start=True, stop=True)

                    def s_fin8():
                        nc.scalar.activation(fs_bc[:, bs], st["bc_ps"],
                                             AF.Copy)
                        nc.vector.tensor_copy(fs_bc16[:, bs], st["bc_ps"])

                    stages.extend([s_fin1, s_fin2, s_fin3, s_fin4, s_fin5,
                                   s_fin6, s_fin7, s_fin8])

                # Uniform compute pitch: pad every group's compute start to a
                # fixed stage index, past the batch finalize of ANY batch
                # (kills batch-lockstep over PSUM tiles and guarantees the
                # fs_bc writes precede all readers in program order).
                target = 13 + (FB - 1) * SKEW
                if len(stages) < target:
                    stages.extend([None] * (target - len(stages)))
                fs_b3 = fs_bc[:, gsl].broadcast_to([N, G, N])
                fs16_b3 = fs_bc16[:, gsl].broadcast_to([N, G, N])

                if num_iters == 0:
                    def s_q0():
                        st["q5s"] = mp.tile([N, G * N], F16, tag="q5s",
                                            bufs=2, name="q5s")
                        nc.vector.tensor_tensor(
                            st["q5s"],
                            eyena.rearrange("p (m c) -> p m c", c=N), fs_b3,
                            op=ALU.mult)
                    stages.append(s_q0)
                else:
                    def s_vt():
                        # vt16 = fs * W^T  (gpsimd, SBUF only)
                        st["vt16"] = mp.tile([N, G * N], F16, tag="vt16",
                                             bufs=3, name="vt16")
                        nc.gpsimd.tensor_tensor(
                            st["vt16"].rearrange("p (m c) -> p m c", c=N),
                            st["wt16"].rearrange("p (m c) -> p m c", c=N),
                            fs16_b3, op=ALU.mult)

                    def s_hmm():
                        st["hps"] = pp.tile([N, G * N], F32, tag="ps",
                                            bufs=3, name="hps")
                        for i in range(G):
                            nc.tensor.matmul(st["hps"][:, sl[i]],
                                             st["vt16"][:, sl[i]],
                                             st["wt16"][:, sl[i]],
                                             start=True, stop=True)

                    def s_h16():
                        st["h16"] = mp.tile([N, G * N], F16, tag="h16", bufs=5,
                                            name="h16")
                        nc.scalar.activation(st["h16"], st["hps"], AF.Copy)

                    stages.extend([s_vt, s_hmm, s_h16])

                if num_iters == 1:
                    def s_q1():
                        st["q16"] = mp.tile([N, G * N], F16, tag="q16", bufs=5,
                                            name="q16")
                        nc.vector.scalar_tensor_tensor(
                            st["q16"], st["h16"], float(-2.0 ** (-2 * A_EXP)),
                            c21a, op0=ALU.mult, op1=ALU.add)
                    stages.append(s_q1)

                if num_iters >= 2:
                    def s_bmm():
                        st["bps"] = pp.tile([N, G * N], F32, tag="ps",
                                            bufs=3, name="bps")
                        hb = G // 2 * N      # per-PSUM-bank split
                        for h in range(2):
                            bsl = slice(h * hb, (h + 1) * hb)
                            nc.tensor.matmul(st["bps"][:, bsl], c4an16,
                                             st["h16"][:, bsl],
                                             start=True, stop=False)
                            for i in range(h * G // 2, (h + 1) * G // 2):
                                nc.tensor.matmul(st["bps"][:, sl[i]],
                                                 st["h16"][:, sl[i]],
                                                 st["h16"][:, sl[i]],
                                                 start=False,
                                                 stop=(i % (G // 2) == G // 2 - 1),
                                                 skip_group_check=True)

                    def s_b16():
                        st["b16"] = mp.tile([N, G * N], F16, tag="b16", bufs=2,
                                            name="b16")
                        nc.vector.scalar_tensor_tensor(
                            st["b16"], st["bps"], -1.0, c6a2n,
                            op0=ALU.mult, op1=ALU.add)

                    def s_cmm():
                        st["cps"] = pp.tile([N, G * N], F32, tag="ps",
                                            bufs=3, name="cps")
                        for i in range(G):
                            nc.tensor.matmul(st["cps"][:, sl[i]],
                                             st["h16"][:, sl[i]],
                                             st["b16"][:, sl[i]],
                                             start=True, stop=True)

                    def s_q2():
                        st["q16"] = mp.tile([N, G * N], F16, tag="q16", bufs=5,
                                            name="q16")
                        nc.vector.scalar_tensor_tensor(
                            st["q16"], st["cps"], float(2.0 ** (-4 * A_EXP)),
                            c4ni, op0=ALU.mult, op1=ALU.add)

                    stages.extend([s_bmm, s_b16, s_cmm, s_q2])

                def make_iter(j, last):
                    # m-evac: first iteration splits matrices ACT (with a
                    # per-matrix -2I PSUM preload) / DVE; the rest are DVE.
                    n_act = M1_ACT if (j == 0 and num_iters > 3) else 0

                    def s_tmm():
                        st["tps"] = pp.tile([N, G * N], F32, tag="ps",
                                            bufs=3, name="tps")
                        for i in range(G):
                            if i < n_act:
                                nc.tensor.matmul(st["tps"][:, sl[i]], c32i16,
                                                 cn2i16[:, sl[i]],
                                                 start=True, stop=False)
                            nc.tensor.matmul(
                                st["tps"][:, sl[i]],
                                st["h16"][:, sl[i]], st["q16"][:, sl[i]],
                                start=(i >= n_act), stop=True,
                                skip_group_check=True)

                    def s_m():
                        st["m16"] = mp.tile([N, G * N], F16, tag="m16", bufs=3,
                                            name="m16")
                        if n_act > 0:
                            nc.scalar.activation(st["m16"][:, 0:n_act * N],
                                                 st["tps"][:, 0:n_act * N],
                                                 AF.Copy, scale=-1.0)
                        if n_act < G:
                            nc.vector.scalar_tensor_tensor(
                                st["m16"][:, n_act * N:],
                                st["tps"][:, n_act * N:], -1.0,
                                twoi[:, n_act * N:],
                                op0=ALU.mult, op1=ALU.add)

                    def s_qmm():
                        st["qps"] = pp.tile([N, G * N], F32, tag="ps",
                                            bufs=3, name="qps")
                        for i in range(G):
                            nc.tensor.matmul(st["qps"][:, sl[i]],
                                             st["q16"][:, sl[i]],
                                             st["m16"][:, sl[i]],
                                             start=True, stop=True)

                    def s_qe():
                        st["q16"] = mp.tile([N, G * N], F16, tag="q16",
                                            bufs=5, name="q16")
                        nc.scalar.activation(st["q16"], st["qps"], AF.Copy)

                    return [s_tmm, s_m, s_qmm, s_qe]

                for j in range(num_iters - 2):
                    stages.extend(make_iter(j, last=(j == num_iters - 3)))

                if num_iters >= 1:
                    def s_ws():
                        # ws16 = fs * W  (gpsimd, SBUF only)
                        st["ws16"] = mp.tile([N, G * N], F16, tag="ws16",
                                             bufs=3, name="ws16")
                        nc.gpsimd.tensor_tensor(
                            st["ws16"].rearrange("p (m c) -> p m c", c=N),
                            st["w16"].rearrange("p (m c) -> p m c", c=N),
                            fs16_b3, op=ALU.mult)
                    stages.append(s_ws)

                def s_xmm():
                    st["xps"] = pp.tile([N, G * N], F32, tag="ps",
                                        bufs=3, name="xps")
                    lhs = st["w16"] if num_iters == 0 else st["ws16"]
                    rhs = st["q5s"] if num_iters == 0 else st["q16"]
                    for i in range(G):
                        nc.tensor.matmul(st["xps"][:, sl[i]],
                                         lhs[:, sl[i]], rhs[:, sl[i]],
                                         start=True, stop=True)

                def s_xout():
                    st["xout"] = mp.tile([N, G * N], F32, tag="xout", bufs=3,
                                         name="xout")
                    nc.scalar.activation(st["xout"], st["xps"], AF.Copy)

                def s_out():
                    nc.sync.dma_start(
                        X3[m0:m0 + G].rearrange("m r c -> r m c"),
                        st["xout"].rearrange("p (m c) -> p m c", c=N))

                stages.extend([s_xmm, s_xout, s_out])
                return stages

            all_stages = [group_stages(g) for g in range(N_GROUPS)]
            S = max(len(s) for s in all_stages)
            for t in range(S + (N_GROUPS - 1) * SKEW):
                for g in range(N_GROUPS):
                    j = t - g * SKEW
                    if 0 <= j < len(all_stages[g]) and all_stages[g][j]:
                        all_stages[g][j]()

    nc.compile()
    return nc


def _get_nc(num_iters: int):
    nc = _nc_cache.get(num_iters)
    if nc is None:
        nc = _build(num_iters)
        _nc_cache[num_iters] = nc
    return nc


def _consts():
    eye = np.eye(N, dtype=np.float32)
    eyeG = np.tile(eye, (1, G))
    a = float(2.0 ** A_EXP)
    return {
        "EYE32": eye,
        "TWOI": 2.0 * eyeG,
        "C6A2N": (-6.0 * a * a) * eyeG,
        "C4NI": (4.0 / a) * eyeG,
        "C4AN16": (-4.0 * a * eye).astype(np.float16),
        "C32I16": (32.0 * eye).astype(np.float16),
        "CN2I16": ((-2.0 / 32.0) * eyeG).astype(np.float16),
        "C21A": (2.0 / a) * eyeG,
        "EYENA": (1.0 / a) * eyeG,
    }


def kernel(W, num_iters, _trace=False, _trace_kwargs=None):
    ni = int(num_iters)
    W = np.ascontiguousarray(np.asarray(W, dtype=np.float32))
    batch_shape = W.shape[:-2]
    Wr = W.reshape(N_CORES, M_PER_CORE, N * N)
    nc = _get_nc(ni)
    consts = _consts()
    import concourse.mybir as _mb
    expected = set()
    for alloc in nc.m.functions[0].allocations:
        if isinstance(alloc, _mb.MemoryLocationSet) and alloc.kind == "ExternalInput":
            expected.add(alloc.memorylocations[0].name)
    consts = {k: v for k, v in consts.items() if k in expected}
    in_maps = [dict(W=Wr[c], **consts) for c in range(N_CORES)]
    res = bass_utils.run_bass_kernel_spmd(
        nc, in_maps, core_ids=list(range(N_CORES)),
        trace=_trace, **(_trace_kwargs or {}))
    X = np.stack([r["X"] for r in res.results])
    X = X.reshape(*batch_shape, N, N)
    if _trace:
        return X, res
    return X
